# revision 1
# baseline (speedup 1.0000x reference)
"""Trainium2 Bass kernel for nn_MemoryBlock (sliding-window attention +
neural long-term memory + gated FFN), distributed over 8 NeuronCores.

Sharding: data-parallel over the flattened token axis. Core c owns a
contiguous 1024-token block (batch c//4, seq block c%4) plus, on tail cores
3/7, the 10 persistent-memory tokens of their batch. Weights are replicated;
the host packs each weight matrix into per-output-block row-major bf16 tiles
(wP[m, p, k*128+c] = W[m*128+c, k*128+p]) so every output block's weights
arrive in ONE large contiguous DMA instead of 8-32 small strided ones.
Activations stay feature-major ([feat, tok]) on chip in bf16 (fast weight
load + full PE rate at any moving width; rel-err ~2e-3 vs the 2e-2 budget);
PSUM accumulation, LN statistics and the softmax stay fp32. The per-token
key normalization is folded into the kp eviction so the Wi matmuls start
the moment x lands. The token-axis grad contraction uses PE transposes of
its two operands (h2, diff — diff is computed feature-major as
predT - kpT). The [M,M] surprise gradient is AllReduce-summed in bf16 as
two column halves so reduce, W_new compose and the memo matmuls pipeline;
window attention overlaps the collective.

SBUF: five tag banks (X/A/B 8x[P,1040], C/D 9x[P,1040]) reused phase to
phase; Tile's slot tracking serializes reuse:
  X0-7: xsT -> wnew -> gateT -> ffs[24:32]
  A0-7: knT -> h1T -> diffT -> xstok[0:8] -> memoT -> ffs[0:8]
  B0-7: kpT -> h2tok[0:8] -> attn kts / LN1 borrow -> y2/x2T
  C0-8: difftok[0:9] -> y1/x1T -> LN3 borrow -> ffs[8:16](C5..)
  D0-8: kn temps -> h2T(D0-7) -> difftok9/xstok8 spill -> moT -> ffs[16:24]
        -> out tiles
"""
import os
import sys

for _p in ("/opt/trn_rl_repo", "/root/.axon_site/_ro/trn_rl_repo"):
    if os.path.isdir(_p) and _p not in sys.path:
        sys.path.insert(0, _p)

import numpy as np

NCORES = 8
B, S, H, FF = 2, 4096, 1024, 4096
WIN, SLOTS = 16, 10
P = 128
TOKX = 1024            # x tokens per core
HALO = 16
EXTRA = 16             # persistent tokens, padded (10 real)
TOKV = TOKX + EXTRA    # 1040 kv tokens per core; also xs length (halo+block)
HB = H // P            # 8
FFB = FF // P          # 32
NKV = B * (S + SLOTS)  # 8212 real kv tokens (grad scaling)
NTT = (TOKV + P - 1) // P  # 9 token tiles
EPS_LN = 1e-5
EPS_NORM = 1e-12
NEG = -1e30
SCL = float(H) ** -0.5

CH_V = [(0, 512), (512, 512), (1024, 16)]   # column chunks for TOKV
CH_X = [(0, 512), (512, 512)]               # column chunks for TOKX
PSTAGS = ["mmA", "mmB", "mmC"]


def _build(native_silu: bool, collective: bool = True):
    import concourse.bass as bass
    import concourse.bacc as bacc
    import concourse.tile as tile
    from concourse import mybir

    F32R = mybir.dt.float32r
    fp32 = mybir.dt.float32
    BF16 = mybir.dt.bfloat16
    AF = mybir.ActivationFunctionType
    A = mybir.AluOpType

    nc = bacc.Bacc("TRN2", target_bir_lowering=False, debug=False,
                   num_devices=NCORES)

    def din(name, shape, dt=fp32):
        return nc.dram_tensor(name, list(shape), dt, kind="ExternalInput")

    xsT_d = din("xsT", (H, TOKV), BF16)
    xs_d = din("xs", (TOKV, H), BF16)
    pT_d = din("pT", (H, EXTRA), BF16)
    wiP_d = din("wiP", (HB, P, H), BF16)
    wl1P_d = din("wl1P", (HB, P, H), BF16)
    wl2P_d = din("wl2P", (HB, P, H), BF16)
    memWP_d = din("memWP", (HB, P, H), BF16)
    woP_d = din("woP", (HB, P, H), BF16)
    wgxP_d = din("wgxP", (HB, P, H), BF16)
    wgmP_d = din("wgmP", (HB, P, H), BF16)
    w1P_d = din("w1P", (FFB, P, H), BF16)
    w2P_d = din("w2P", (HB, P, FF), BF16)
    wkeepT_d = din("wkeepT", (H, H), BF16)
    bcols_d = {}
    for nm in ("bi", "bl1", "bl2", "memb", "bo", "bg", "b2",
               "g1", "be1", "g2", "be2", "g3", "be3"):
        bcols_d[nm] = din(nm + "_c", (P, HB), fp32)
    b1c_d = din("b1_c", (P, FFB), fp32)
    cg_d = din("cg_c", (P, 1), fp32)
    mask0_d = din("mask0", (P, 256))
    maskr_d = din("maskr", (P, 256))
    ident_d = din("ident", (P, P), BF16)
    onescol_d = din("onescol", (P, 1))
    onescolb_d = din("onescolb", (P, 1), BF16)
    onesrow_d = din("onesrow", (1, P))
    zeros_d = din("zeros512", (P, 512), BF16)

    outT_d = nc.dram_tensor("outT", [H, TOKX], fp32, kind="ExternalOutput")

    with tile.TileContext(nc) as tc:
        with (
            nc.allow_low_precision(reason="float32r is bit-identical fp32"),
            tc.tile_pool(name="const", bufs=1) as const,
            tc.tile_pool(name="wpool", bufs=6) as wpool,
            tc.tile_pool(name="acts", bufs=1) as acts,
            tc.tile_pool(name="ps", bufs=1, space="PSUM") as ps,
            tc.tile_pool(name="scratch", bufs=1) as scratch,
            tc.tile_pool(name="dram", bufs=1, space="DRAM") as dram,
        ):
            # ---------- xs + critical-path constants only ----------
            # The Sync engine processes dma_starts in FIFO order (~0.6us
            # each); only what the kn-stats + kp phase needs is emitted
            # before it, so the first weight block issues within ~20 slots
            # instead of ~48. Everything else is emitted after kp.
            xsT = []
            for k in range(HB):
                t = acts.tile([P, TOKV], BF16, tag=f"X{k}", name=f"xsT{k}")
                nc.sync.dma_start(t[:], xsT_d[k * P:(k + 1) * P, :])
                xsT.append(t)
            ones_col_t = const.tile([P, 1], F32R, tag="ones_col")
            nc.sync.dma_start(ones_col_t[:], onescol_d[:, :].bitcast(F32R))
            ones_col = ones_col_t[:]
            ones_row_t = const.tile([1, P], F32R, tag="ones_row")
            nc.sync.dma_start(ones_row_t[:], onesrow_d[:, :].bitcast(F32R))
            ones_row = ones_row_t[:]
            bc = {}
            t = const.tile([P, HB], fp32, tag="bc_bi")
            nc.sync.dma_start(t[:], bcols_d["bi"][:, :])
            bc["bi"] = t
            pT = []
            for k in range(HB):
                t = const.tile([P, EXTRA], BF16, tag=f"pT{k}")
                nc.sync.dma_start(t[:], pT_d[k * P:(k + 1) * P, :])
                pT.append(t)

            # ---------- helpers ----------
            name_ctr = [0]

            def uname(pfx):
                name_ctr[0] += 1
                return f"{pfx}_{name_ctr[0]}"

            def bank(tag, shape, dt=BF16):
                return acts.tile(shape, dt, tag=tag, name=uname(tag))

            def psum_mm(ci):
                return ps.tile([P, 512], fp32, tag=PSTAGS[ci],
                               bufs=(1 if ci == 2 else 2), name=uname("pmm"))

            def psum_row(tag, bufs):
                return ps.tile([1, 512], fp32, tag=tag, bufs=bufs,
                               name=uname(tag))

            def psum_tp():
                return ps.tile([P, P], BF16, tag="st1", bufs=2,
                               name=uname("tp"))

            def s512():
                return scratch.tile([P, 512], F32R, tag="s512", bufs=4,
                                    name=uname("s512"))

            def s512b():
                return scratch.tile([P, 512], BF16, tag="s512b", bufs=4,
                                    name=uname("s512b"))

            def evict_copy(dst, src, bias=0.0, scale=1.0):
                func = AF.Copy if isinstance(bias, float) else AF.Identity
                nc.scalar.activation(dst, src, func, bias=bias, scale=scale)

            def evict_silu(dst, src, bias):
                if native_silu:
                    nc.scalar.activation(dst, src, AF.Silu, bias=bias)
                else:
                    t = s512()
                    w = dst.shape[-1]
                    nc.scalar.activation(t[:, :w], src, AF.Sigmoid, bias=bias)
                    nc.scalar.activation(dst, src, AF.Identity, bias=bias)
                    nc.vector.tensor_mul(dst, dst, t[:, :w])

            def wblk(wp_dram, m, nk, tag="wblk"):
                """One DMA bringing in the whole [P, nk*P] weight row-block
                for output block m (host-packed contiguous)."""
                wt = wpool.tile([P, nk * P], BF16, tag=tag, name=uname("wb"))
                nc.sync.dma_start(wt[:], wp_dram[m, :, :nk * P])
                return wt

            def mmTp(wp_dram, rhs_tiles, nk, chunks, evict, out_tags, ncols):
                """out[m] = evict(sum_k wp[m][k].T @ rhs_tiles[k]); packed
                weights, one DMA per output block."""
                outs = []
                for m in range(len(out_tags)):
                    wt = wblk(wp_dram, m, nk)
                    psums = [psum_mm(ci) for ci in range(len(chunks))]
                    for k in range(nk):
                        for ci, (c0, cw) in enumerate(chunks):
                            nc.tensor.matmul(
                                psums[ci][:, :cw], wt[:, k * P:(k + 1) * P],
                                rhs_tiles[k][:, c0:c0 + cw],
                                start=(k == 0), stop=(k == nk - 1))
                    t = bank(out_tags[m], [P, ncols])
                    for ci, (c0, cw) in enumerate(chunks):
                        evict(t[:, c0:c0 + cw], psums[ci][:, :cw], m)
                    outs.append(t)
                return outs

            def transpose_to_tokmajor(src_tiles, tags):
                """feature-major [P,TOKV] x HB -> token-major [P,H] x NTT."""
                outs = []
                for j in range(NTT):
                    t0 = j * P
                    tw = min(P, TOKV - t0)
                    t = (tags[j] if not isinstance(tags[j], str)
                         else bank(tags[j], [P, H]))
                    for k in range(HB):
                        pt = psum_tp()
                        nc.tensor.transpose(
                            pt[:tw, :], src_tiles[k][:, t0:t0 + tw], ident)
                        evict_copy(t[:tw, k * P:(k + 1) * P], pt[:tw, :])
                    outs.append(t)
                return outs

            def layernorm(y_tiles, ncols, chunks, g_col, be_col, out_tiles,
                          borrow):
                """Feature-axis LN (feature-major layout); final y*g+be into
                out_tiles (may alias y_tiles). borrow = 5 bank tags.
                Chunk-pipelined: each token chunk runs stats->apply fully so
                downstream matmuls on chunk 0 can overlap chunk 1."""
                mean = bank(borrow[0], [1, ncols], F32R)
                rs = bank(borrow[1], [1, ncols], F32R)
                m2 = bank(borrow[2], [1, ncols], F32R)
                mean_b = bank(borrow[3], [P, ncols], F32R)
                rs_b = bank(borrow[4], [P, ncols], F32R)
                for ci, (c0, cw) in enumerate(chunks):
                    s1p = psum_row("st1", 2)
                    s2p = psum_row("st2", 1)
                    for k in range(HB):
                        sq = s512()
                        nc.vector.tensor_mul(sq[:, :cw],
                                             y_tiles[k][:, c0:c0 + cw],
                                             y_tiles[k][:, c0:c0 + cw])
                        nc.tensor.matmul(s1p[:, :cw], ones_colb,
                                         y_tiles[k][:, c0:c0 + cw],
                                         start=(k == 0), stop=(k == HB - 1))
                        nc.tensor.matmul(s2p[:, :cw], ones_col, sq[:, :cw],
                                         start=(k == 0), stop=(k == HB - 1))
                    mc = mean[:, c0:c0 + cw]
                    rc = rs[:, c0:c0 + cw]
                    nc.vector.tensor_scalar_mul(mc, s1p[:, :cw], 1.0 / H)
                    nc.vector.tensor_scalar_mul(rc, s2p[:, :cw], 1.0 / H)
                    nc.vector.tensor_mul(m2[:, c0:c0 + cw], mc, mc)
                    nc.vector.tensor_sub(rc, rc, m2[:, c0:c0 + cw])
                    nc.vector.tensor_scalar_add(rc, rc, EPS_LN)
                    nc.scalar.activation(rc, rc, AF.Sqrt)
                    nc.vector.reciprocal(rc, rc)
                    for src, dst in ((mean, mean_b), (rs, rs_b)):
                        pb = psum_mm(2)
                        nc.tensor.matmul(pb[:, :cw], ones_row,
                                         src[:, c0:c0 + cw],
                                         start=True, stop=True)
                        evict_copy(dst[:, c0:c0 + cw], pb[:, :cw])
                    for k in range(HB):
                        nc.vector.tensor_sub(y_tiles[k][:, c0:c0 + cw],
                                             y_tiles[k][:, c0:c0 + cw],
                                             mean_b[:, c0:c0 + cw])
                        nc.vector.tensor_mul(y_tiles[k][:, c0:c0 + cw],
                                             y_tiles[k][:, c0:c0 + cw],
                                             rs_b[:, c0:c0 + cw])
                        nc.vector.tensor_scalar(
                            out_tiles[k][:, c0:c0 + cw],
                            y_tiles[k][:, c0:c0 + cw],
                            g_col[:, k:k + 1], be_col[:, k:k + 1],
                            op0=A.mult, op1=A.add)
                return out_tiles

            # ---------- kn: row-normalized kv (feature-major) ----------
            rsn = bank("D1", [1, TOKV], F32R)
            for ci, (c0, cw) in enumerate(CH_V):
                ssp = psum_row("st1", 2)
                for k in range(HB):
                    sq = s512()
                    if c0 < TOKX:
                        nc.vector.tensor_mul(
                            sq[:, :cw],
                            xsT[k][:, HALO + c0:HALO + c0 + cw],
                            xsT[k][:, HALO + c0:HALO + c0 + cw])
                    else:
                        nc.vector.tensor_mul(sq[:, :cw], pT[k][:], pT[k][:])
                    nc.tensor.matmul(ssp[:, :cw], ones_col, sq[:, :cw],
                                     start=(k == 0), stop=(k == HB - 1))
                nc.scalar.activation(rsn[:, c0:c0 + cw], ssp[:, :cw], AF.Sqrt)
            rsn_b = bank("D0", [P, TOKV], F32R)
            for ci, (c0, cw) in enumerate(CH_V):
                rc = rsn[:, c0:c0 + cw]
                nc.vector.tensor_scalar_max(rc, rc, EPS_NORM)
                nc.vector.reciprocal(rc, rc)
                pb = psum_mm(2)
                nc.tensor.matmul(pb[:, :cw], ones_row, rc,
                                 start=True, stop=True)
                evict_copy(rsn_b[:, c0:c0 + cw], pb[:, :cw])

            # ---------- memory chain (all feature-major) ----------
            # kp = (x_raw @ Wi.T) * (1/||x||) + bi: the per-token normal-
            # ization is folded into the eviction, so the kp matmuls start
            # as soon as xsT lands (concurrent with the norm stats).
            kpT = []
            for m in range(HB):
                wt = wblk(wiP_d, m, HB)
                psums = [psum_mm(ci) for ci in range(len(CH_V))]
                for k in range(HB):
                    for ci, (c0, cw) in enumerate(CH_V):
                        rhs = (xsT[k][:, HALO + c0:HALO + c0 + cw]
                               if c0 < TOKX else pT[k][:])
                        nc.tensor.matmul(psums[ci][:, :cw],
                                         wt[:, k * P:(k + 1) * P], rhs,
                                         start=(k == 0), stop=(k == HB - 1))
                t = bank(f"B{m}", [P, TOKV])
                for ci, (c0, cw) in enumerate(CH_V):
                    nc.vector.tensor_mul(t[:, c0:c0 + cw], psums[ci][:, :cw],
                                         rsn_b[:, c0:c0 + cw])
                    nc.vector.tensor_scalar_add(t[:, c0:c0 + cw],
                                                t[:, c0:c0 + cw],
                                                bc["bi"][:, m:m + 1])
                kpT.append(t)

            # ---------- deferred constants + attention operands ----------
            # (first readers are all >=40us in; queueing them here keeps the
            # startup Sync FIFO short)
            ident_t = const.tile([P, P], BF16, tag="ident")
            nc.sync.dma_start(ident_t[:], ident_d[:, :])
            ident = ident_t[:]
            ones_colb_t = const.tile([P, 1], BF16, tag="ones_colb")
            nc.sync.dma_start(ones_colb_t[:], onescolb_d[:, :])
            ones_colb = ones_colb_t[:]
            for nm in bcols_d:
                if nm == "bi":
                    continue
                t = const.tile([P, HB], fp32, tag=f"bc_{nm}")
                nc.sync.dma_start(t[:], bcols_d[nm][:, :])
                bc[nm] = t
            b1c = const.tile([P, FFB], fp32, tag="b1c")
            nc.sync.dma_start(b1c[:], b1c_d[:, :])
            cg = const.tile([P, 1], fp32, tag="cg")
            nc.sync.dma_start(cg[:], cg_d[:, :])
            mask0 = const.tile([P, 256], F32R, tag="mask0")
            nc.sync.dma_start(mask0[:], mask0_d[:, :].bitcast(F32R))
            maskr = const.tile([P, 256], F32R, tag="maskr")
            nc.sync.dma_start(maskr[:], maskr_d[:, :].bitcast(F32R))
            zeros_t = const.tile([P, 512], BF16, tag="zeros512")
            nc.sync.dma_start(zeros_t[:], zeros_d[:, :])
            xstok = []
            for j in range(NTT):
                tw = min(P, TOKV - j * P)
                t = bank(f"S{j}", [P, H])
                nc.sync.dma_start(t[:tw, :], xs_d[j * P:j * P + tw, :])
                xstok.append(t)

            h1T = mmTp(wl1P_d, kpT, HB, CH_V,
                       lambda d, s, m: evict_silu(d, s, bc["bl1"][:, m:m + 1]),
                       [f"A{m}" for m in range(HB)], TOKV)

            h2T = mmTp(wl2P_d, h1T, HB, CH_V,
                       lambda d, s, m: evict_silu(d, s, bc["bl2"][:, m:m + 1]),
                       [f"D{m}" for m in range(HB)], TOKV)

            # h2tok transposes depend only on h2T, so they are emitted before
            # the pred/diff matmuls and overlap them on the PE
            h2tok = transpose_to_tokmajor(
                h2T, [f"C{j}" for j in range(NTT)])

            # diffT = (pred + memb) - (kp + bi), all feature-major
            diffT = mmTp(memWP_d, h2T, HB, CH_V,
                         lambda d, s, m: evict_copy(
                             d, s, bias=bc["memb"][:, m:m + 1]),
                         [f"A{m}" for m in range(HB)], TOKV)
            for m in range(HB):
                for c0, cw in CH_V:
                    nc.vector.tensor_sub(diffT[m][:, c0:c0 + cw],
                                         diffT[m][:, c0:c0 + cw],
                                         kpT[m][:, c0:c0 + cw])

            dtk8 = scratch.tile([P, H], BF16, tag="dtk8", bufs=1,
                                name=uname("dtk8"))
            difftok = transpose_to_tokmajor(
                diffT, [f"B{j}" for j in range(HB)] + [dtk8])

            # grad: gradT[n,m] = cg * sum_t h2tok[t,n] * difftok[t,m].
            # Two bf16 AllReduces over column halves: half 0 is in flight
            # while half 1's matmuls run, and W_new/memo consumers of half 0
            # start before half 1 lands.
            gins = [dram.tile([H, 512], BF16, name=f"gin{i}")
                    for i in range(2)]
            gouts = [dram.tile([H, 512], BF16, addr_space="Shared",
                               name=f"gout{i}") for i in range(2)]
            for ci, (c0, cw) in enumerate(CH_X):
                for a in range(HB):
                    pp = psum_mm(ci)
                    for j in range(NTT):
                        tw = min(P, TOKV - j * P)
                        nc.tensor.matmul(pp[:, :cw],
                                         h2tok[j][:tw, a * P:(a + 1) * P],
                                         difftok[j][:tw, c0:c0 + cw],
                                         start=(j == 0), stop=(j == NTT - 1))
                    g = scratch.tile([P, 512], BF16, tag="g16", bufs=2,
                                     name=uname("g16"))
                    nc.scalar.activation(g[:, :cw], pp[:, :cw], AF.Copy,
                                         scale=cg[:, :1])
                    nc.sync.dma_start(gins[ci][a * P:(a + 1) * P, :cw],
                                      g[:, :cw])
                if collective:
                    nc.gpsimd.collective_compute(
                        "AllReduce", A.add,
                        replica_groups=[list(range(NCORES))],
                        ins=[gins[ci][:].opt()], outs=[gouts[ci][:].opt()],
                    )
                else:
                    nc.sync.dma_start(gouts[ci][:, :], gins[ci][:, :])

            # ---------- attention (overlaps the collective; its tiles are
            # dedicated so no WAR serialization against the grad phase) ----
            y1 = [bank(f"Y{k}", [P, TOKX]) for k in range(HB)]

            for qg in range(2):  # query groups of 512
                kts = []
                for kt in range(5):
                    t = scratch.tile([P, 512], BF16, tag="kts", bufs=5,
                                     name=uname("kts"))
                    nc.vector.tensor_copy(t[:], zeros_t[:])
                    kts.append(t)
                for sj in range(4):  # 128-query subtiles
                    qt = qg * 4 + sj
                    kw = 144 if qt == 7 else 256  # key-window width
                    pp = psum_mm(0)
                    for k in range(HB):
                        nc.tensor.matmul(
                            pp[:, :kw],
                            xsT[k][:, HALO + qt * P:HALO + qt * P + P],
                            xsT[k][:, qt * P:qt * P + kw],
                            start=(k == 0), stop=(k == HB - 1))
                    probs = s512b()
                    msk = mask0 if qt == 0 else maskr
                    nc.vector.tensor_add(probs[:, :kw], pp[:, :kw],
                                         msk[:, :kw])
                    mx = scratch.tile([P, 1], fp32, tag="mx", bufs=3,
                                      name=uname("mx"))
                    nc.vector.reduce_max(mx[:], probs[:, :kw],
                                         axis=mybir.AxisListType.X)
                    nc.vector.tensor_scalar_mul(mx[:], mx[:], -SCL)
                    nc.scalar.activation(probs[:, :kw], probs[:, :kw],
                                         AF.Exp, bias=mx[:, :1], scale=SCL)
                    sm = scratch.tile([P, 1], fp32, tag="sm", bufs=3,
                                      name=uname("sm"))
                    nc.vector.reduce_sum(sm[:], probs[:, :kw],
                                         axis=mybir.AxisListType.X)
                    nc.vector.reciprocal(sm[:], sm[:])
                    nc.vector.tensor_scalar_mul(probs[:, :kw], probs[:, :kw],
                                                sm[:, :1])
                    for half in range(2):
                        hw_ = min(P, kw - half * P)
                        pt = psum_tp()
                        nc.tensor.transpose(
                            pt[:hw_, :],
                            probs[:, half * P:half * P + hw_], ident)
                        evict_copy(
                            kts[sj + half][:hw_, sj * P:(sj + 1) * P],
                            pt[:hw_, :])
                for k in range(HB):
                    pp = psum_mm(1)
                    for kt in range(5):
                        ktw = min(P, TOKV - (qg * 4 + kt) * P)
                        nc.tensor.matmul(
                            pp[:],
                            xstok[qg * 4 + kt][:ktw, k * P:(k + 1) * P],
                            kts[kt][:ktw, :],
                            start=(kt == 0), stop=(kt == 4))
                    nc.vector.tensor_add(
                        y1[k][:, qg * 512:(qg + 1) * 512], pp[:],
                        xsT[k][:, HALO + qg * 512:HALO + (qg + 1) * 512])

            x1T = layernorm(y1, TOKX, CH_X, bc["g1"], bc["be1"], y1,
                            ["B0", "B1", "B2", "B3", "B4"])

            # ---------- W_new compose (per half, pipelined with collective)
            wnew = []
            for k in range(HB):
                t = bank(f"X{k}", [P, H])
                nc.sync.dma_start(t[:], wkeepT_d[k * P:(k + 1) * P, :])
                for ci, (c0, cw) in enumerate(CH_X):
                    gb = scratch.tile([P, 512], BF16, tag="gr16", bufs=3,
                                      name=uname("gr16"))
                    nc.sync.dma_start(gb[:, :cw],
                                      gouts[ci][k * P:(k + 1) * P, :cw])
                    nc.vector.tensor_add(t[:, c0:c0 + cw], t[:, c0:c0 + cw],
                                         gb[:, :cw])
                wnew.append(t)

            # ---------- memoT = W_new @ h2T + memb ----------
            # (the persistent-token columns of mem_out are discarded by the
            # reference's [:, :s] slice, so memo/mo only cover TOKX tokens)
            memoT = []
            for m in range(HB):
                psums = [psum_mm(ci) for ci in range(len(CH_X))]
                for k in range(HB):
                    for ci, (c0, cw) in enumerate(CH_X):
                        nc.tensor.matmul(psums[ci][:, :cw],
                                         wnew[k][:, m * P:(m + 1) * P],
                                         h2T[k][:, c0:c0 + cw],
                                         start=(k == 0), stop=(k == HB - 1))
                t = bank(f"A{m}", [P, TOKX])
                for ci, (c0, cw) in enumerate(CH_X):
                    evict_copy(t[:, c0:c0 + cw], psums[ci][:, :cw],
                               bias=bc["memb"][:, m:m + 1])
                memoT.append(t)

            # ---------- moT = Wo @ memoT + bo ----------
            moT = mmTp(woP_d, memoT, HB, CH_X,
                       lambda d, s, m: evict_copy(
                           d, s, bias=bc["bo"][:, m:m + 1]),
                       [f"D{m}" for m in range(HB)], TOKX)

            # ---------- gate = sigmoid(Wgx@x1 + Wgm@mo + bg) ----------
            gateT = []
            for m in range(HB):
                psums = [psum_mm(ci) for ci in range(len(CH_X))]
                for half, (wd, rhs) in enumerate(
                        ((wgxP_d, x1T), (wgmP_d, moT))):
                    wt = wblk(wd, m, HB)
                    for k in range(HB):
                        for ci, (c0, cw) in enumerate(CH_X):
                            nc.tensor.matmul(
                                psums[ci][:, :cw], wt[:, k * P:(k + 1) * P],
                                rhs[k][:, c0:c0 + cw],
                                start=(half == 0 and k == 0),
                                stop=(half == 1 and k == HB - 1))
                t = bank(f"X{m}", [P, TOKX])
                for ci, (c0, cw) in enumerate(CH_X):
                    nc.scalar.activation(t[:, c0:c0 + cw], psums[ci][:, :cw],
                                         AF.Sigmoid,
                                         bias=bc["bg"][:, m:m + 1])
                gateT.append(t)

            # ---------- combine + LN2 -> x2T ----------
            y2 = []
            for k in range(HB):
                d = bank("A6" if k % 2 == 0 else "A7", [P, TOKX])
                t = bank(f"B{k}", [P, TOKX])
                for c0, cw in CH_X:
                    c = slice(c0, c0 + cw)
                    nc.vector.tensor_sub(d[:, c], x1T[k][:, c], moT[k][:, c])
                    nc.vector.tensor_mul(d[:, c], d[:, c], gateT[k][:, c])
                    nc.vector.tensor_add(t[:, c], x1T[k][:, c], moT[k][:, c])
                    nc.vector.tensor_add(t[:, c], t[:, c], d[:, c])
                y2.append(t)
            x2T = layernorm(y2, TOKX, CH_X, bc["g2"], bc["be2"], y2,
                            ["A0", "A1", "A2", "A3", "A4"])

            # ---------- FFN ----------
            ffs_tags = ([f"A{j}" for j in range(8)]
                        + [f"C{j}" for j in range(8)]
                        + [f"D{j}" for j in range(8)]
                        + [f"X{k}" for k in range(8)])
            ffsT = mmTp(w1P_d, x2T, HB, CH_X,
                        lambda d, s, m: evict_silu(d, s, b1c[:, m:m + 1]),
                        ffs_tags, TOKX)

            # ff2 + residual accumulated into x2T; W2 row-blocks arrive in
            # four [P, 1024] quarter-DMAs per output block
            for m in range(HB):
                psums = [psum_mm(ci) for ci in range(len(CH_X))]
                for q in range(4):
                    wt = wpool.tile([P, H], BF16, tag="wblk",
                                    name=uname("wb2"))
                    nc.sync.dma_start(
                        wt[:], w2P_d[m, :, q * H:(q + 1) * H])
                    for kk in range(8):
                        k = q * 8 + kk
                        for ci, (c0, cw) in enumerate(CH_X):
                            nc.tensor.matmul(psums[ci][:, :cw],
                                             wt[:, kk * P:(kk + 1) * P],
                                             ffsT[k][:, c0:c0 + cw],
                                             start=(k == 0),
                                             stop=(k == FFB - 1))
                for ci, (c0, cw) in enumerate(CH_X):
                    ft = s512()
                    nc.scalar.activation(ft[:, :cw], psums[ci][:, :cw],
                                         AF.Identity,
                                         bias=bc["b2"][:, m:m + 1])
                    nc.vector.tensor_add(x2T[m][:, c0:c0 + cw],
                                         x2T[m][:, c0:c0 + cw], ft[:, :cw])

            # ---------- LN3 -> outT ----------
            outt = [bank(f"D{k % 3}", [P, TOKX], F32R) for k in range(HB)]
            out3 = layernorm(x2T, TOKX, CH_X, bc["g3"], bc["be3"], outt,
                             ["C0", "C1", "C2", "C3", "C4"])
            for k in range(HB):
                for c0, cw in CH_X:
                    nc.sync.dma_start(
                        outT_d[k * P:(k + 1) * P, c0:c0 + cw].bitcast(F32R),
                        out3[k][:, c0:c0 + cw])

    nc.compile()
    return nc


_NC_CACHE = {}


def _get_nc(native_silu: bool, collective: bool = True):
    key = (bool(native_silu), bool(collective))
    if key not in _NC_CACHE:
        _NC_CACHE[key] = _build(native_silu, collective)
    return _NC_CACHE[key]


def _bf16():
    import ml_dtypes
    return ml_dtypes.bfloat16


def _pack_w(w):
    """[out, in] weight -> [out/P, P, in] bf16 with wP[m, p, k*P+c] =
    W[m*P+c, k*P+p]: each output block's lhsT tiles contiguous."""
    f = np.float32
    wT = np.asarray(w, f).T                      # [K, M]
    K, M = wT.shape
    return np.ascontiguousarray(
        wT.reshape(K // P, P, M // P, P).transpose(2, 1, 0, 3)
        .reshape(M // P, P, K).astype(_bf16()))


def _host_prep(inputs):
    """Build the 8 per-core input maps from the full problem inputs."""
    f = np.float32
    x = np.asarray(inputs["x"], f)
    Pm = np.asarray(inputs["P"], f)
    fgate = float(np.asarray(inputs["fgate"]).reshape(-1)[0])
    lrate = float(np.asarray(inputs["lrate"]).reshape(-1)[0])

    shared = {
        "wiP": _pack_w(inputs["Wi"]), "wl1P": _pack_w(inputs["Wl1"]),
        "wl2P": _pack_w(inputs["Wl2"]), "memWP": _pack_w(inputs["mem_W"]),
        "woP": _pack_w(inputs["Wo"]),
        "wgxP": _pack_w(np.asarray(inputs["Wg"], f)[:, :H]),
        "wgmP": _pack_w(np.asarray(inputs["Wg"], f)[:, H:]),
        "w1P": _pack_w(inputs["W1"]), "w2P": _pack_w(inputs["W2"]),
        "wkeepT": np.ascontiguousarray(
            (1.0 - fgate) * np.asarray(inputs["mem_W"], f).T).astype(_bf16()),
        "cg_c": np.full((P, 1), 2.0 * lrate / (NKV * H), f),
        "b1_c": np.ascontiguousarray(
            np.asarray(inputs["b1"], f).reshape(FFB, P).T),
    }
    for nm, key in (("bi", "bi"), ("bl1", "bl1"), ("bl2", "bl2"),
                    ("memb", "mem_b"), ("bo", "bo"), ("bg", "bg"),
                    ("b2", "b2"), ("g1", "g1"), ("be1", "be1"),
                    ("g2", "g2"), ("be2", "be2"), ("g3", "g3"),
                    ("be3", "be3")):
        shared[nm + "_c"] = np.ascontiguousarray(
            np.asarray(inputs[key], f).reshape(HB, P).T)

    r = np.arange(P)[:, None]
    c = np.arange(256)[None, :]
    band = (c >= r) & (c <= r + WIN)
    shared["maskr"] = np.where(band, 0.0, NEG).astype(f)
    shared["mask0"] = np.where(band & (c >= HALO), 0.0, NEG).astype(f)
    bf = _bf16()
    shared["ident"] = np.eye(P, dtype=bf)
    shared["onescol"] = np.ones((P, 1), f)
    shared["onescolb"] = np.ones((P, 1), bf)
    shared["onesrow"] = np.ones((1, P), f)
    shared["zeros512"] = np.zeros((P, 512), bf)

    in_maps = []
    for core in range(NCORES):
        b, blk = divmod(core, 4)
        t0 = blk * TOKX
        xs = np.zeros((TOKV, H), f)
        xs[HALO:HALO + TOKX] = x[b, t0:t0 + TOKX]
        if blk > 0:
            xs[:HALO] = x[b, t0 - HALO:t0]
        pT = np.zeros((H, EXTRA), f)
        if blk == 3:
            pT[:, :SLOTS] = Pm.T
        m = dict(shared)
        m["xs"] = xs.astype(bf)
        m["xsT"] = np.ascontiguousarray(xs.T).astype(bf)
        m["pT"] = pT.astype(bf)
        in_maps.append(m)
    return in_maps


def _assemble(results):
    out = np.empty((B, S, H), np.float32)
    for core in range(NCORES):
        b, blk = divmod(core, 4)
        out[b, blk * TOKX:(blk + 1) * TOKX, :] = \
            np.asarray(results[core]["outT"]).T
    return out


def kernel(**inputs) -> np.ndarray:
    from concourse.bass_utils import run_bass_kernel_spmd
    native = os.environ.get("MEMBLK_NATIVE_SILU", "1") == "1"
    nc = _get_nc(native)
    in_maps = _host_prep(inputs)
    res = run_bass_kernel_spmd(nc, in_maps, list(range(NCORES)))
    return _assemble(res.results)


def kernel_sim(**inputs) -> np.ndarray:
    """CoreSim path for correctness validation (no hardware)."""
    from concourse.bass_interp import MultiCoreSim
    nc = _get_nc(False)
    in_maps = _host_prep(inputs)
    sim = MultiCoreSim(nc, NCORES)
    for i in range(NCORES):
        for k, v in in_maps[i].items():
            sim.cores[i].tensor(k)[:] = v
    sim.simulate(check_with_hw=False)
    results = [{"outT": np.array(sim.cores[i].tensor("outT"))}
               for i in range(NCORES)]
    return _assemble(results)

